# revision 9
# baseline (speedup 1.0000x reference)
"""Trainium2 Bass kernel for CumulantSOAP_CV.

reference math:
    m    = mean(X, axis=0)                       # (576,)
    mom1 = mean(X - m, axis=0)  (~0)             # (576,)
    mom2 = mean((X - m)^2, axis=0)               # (576,)
    cum  = interleave(m, mom1, mom2)             # (1, 1728)
    out  = (cum - mu) @ W                        # (1, 4)

Device kernel (SPMD over 8 cores, X sharded row-wise): each core streams its
(25000, 576) shard from HBM once and produces the per-shard column sums
S1 = sum(x) and S2 = sum(x^2) as a (1, 1152) output.  The tiny moment/
projection math on the 8 partial vectors happens on host in float64.

Per-core layout: the shard is zero-padded to 25088 rows and viewed as 28
chunks of (128 partitions x 4032 free), i.e. each partition holds 7
consecutive rows, so every chunk is a single contiguous 2.06 MB HBM read.
128 partitions matters: the HWDGE splits a DMA across SDMA engines by
divisibility of the partition count (125 partitions -> only 5 engines at
~23 GB/s each = 117 GB/s; 128 -> all 16 engines = full ~360 GB/s).
Column sums reduce over the partition axis with a ones(128,1).T @ tile bf16
matmul (1 cycle/row), accumulated in PSUM (fp32) across all row-groups.
X^2 comes from one ScalarE Square activation per chunk (bf16 out); the bf16
copy of X comes from one VectorE tensor_copy per chunk.  bf16 rounding of
the summands perturbs the final (1,4) projection by ~1e-6 relative -- far
inside tolerance.  Padded zero rows contribute nothing to either sum.
"""

import sys
import types

import numpy as np

N_CORES = 8
N_ROWS = 200000
P = 576
PROJ_DIMS = 4
ROWS_PER_CORE = N_ROWS // N_CORES   # 25000
PART = 128                          # SBUF partitions used per tile
RPP = 7                             # rows per partition per chunk
CHUNK_ROWS = PART * RPP             # 896
CHUNKS = 28                         # per core (28*896 = 25088, padded)
PAD_ROWS = CHUNKS * CHUNK_ROWS      # 25088
FREE = RPP * P                      # 4032
SPLIT = 288                         # 576 = 2x288; each half fits one PSUM bank


def _build(chunks=CHUNKS, part=PART, rpp=RPP):
    import concourse.bacc as bacc
    import concourse.mybir as mybir
    import concourse.tile as tile

    free = rpp * P
    nc = bacc.Bacc(None, target_bir_lowering=False)
    f32 = mybir.dt.float32
    bf16 = mybir.dt.bfloat16
    x = nc.dram_tensor("x", [chunks, part, free], f32, kind="ExternalInput")
    out = nc.dram_tensor("out", [1, 2 * P], f32, kind="ExternalOutput")

    with tile.TileContext(nc) as tc:
        with (
            tc.tile_pool(name="xp", bufs=6) as xp,
            tc.tile_pool(name="xmp", bufs=3) as xmp,
            tc.tile_pool(name="sqp", bufs=3) as sqp,
            tc.tile_pool(name="cst", bufs=1) as cst,
            tc.tile_pool(name="op", bufs=1) as op,
            tc.tile_pool(name="ps", bufs=1, space="PSUM") as ps,
        ):
            ones = cst.tile([part, 1], bf16)
            nc.vector.memset(ones[:], 1.0)
            # acc[0:2] = S1 halves, acc[2:4] = S2 halves
            acc = [
                ps.tile([1, SPLIT], f32, name=f"acc{i}", tag=f"acc{i}")
                for i in range(4)
            ]

            # Split the final chunk column-wise so the post-last-DMA compute
            # tail (cast + square + matmuls on a full 4032-wide tile) shrinks.
            pieces = [(c, 0, rpp) for c in range(chunks - 1)]
            pieces += [(chunks - 1, 0, rpp - 2), (chunks - 1, rpp - 2, rpp)]
            for c, g0, g1 in pieces:
                w = (g1 - g0) * P
                xt = xp.tile([part, w], f32, name="xt", tag="xt")
                nc.sync.dma_start(out=xt[:], in_=x[c][:, g0 * P:g1 * P])
                xm = xmp.tile([part, w], bf16, name="xm", tag="xm")
                nc.vector.tensor_copy(xm[:], xt[:])
                sq = sqp.tile([part, w], bf16, name="sq", tag="sq")
                nc.scalar.square(sq[:], xt[:])
                for g in range(g1 - g0):
                    o = g * P
                    first = c == 0 and g0 == 0 and g == 0
                    last = c == chunks - 1 and g0 + g == rpp - 1
                    for h in range(2):
                        s = o + h * SPLIT
                        nc.tensor.matmul(
                            acc[h][:],
                            ones[:],
                            xm[:, s:s + SPLIT],
                            start=first,
                            stop=last,
                        )
                        nc.tensor.matmul(
                            acc[2 + h][:],
                            ones[:],
                            sq[:, s:s + SPLIT],
                            start=first,
                            stop=last,
                        )

            ot = op.tile([1, 2 * P], f32)
            for h in range(2):
                nc.vector.tensor_copy(ot[:, h * SPLIT:(h + 1) * SPLIT], acc[h][:])
                nc.vector.tensor_copy(
                    ot[:, P + h * SPLIT:P + (h + 1) * SPLIT], acc[2 + h][:]
                )
            nc.sync.dma_start(out=out[:], in_=ot[:])
    nc.compile()
    return nc


def _install_ntff_hook():
    """This image's antenv lacks axon_hooks, which bass_utils imports when
    tracing is requested (trace=True or BASS_TRACE=1).  Recreate the module
    from the injected libaxon_pjrt.so so tracing works instead of crashing.
    Harmless when tracing is off."""
    try:
        import antenv.axon_hooks  # noqa: F401
        return
    except ImportError:
        pass
    try:
        import antenv
        import trn_agent_boot.trn_boot as tb

        hook = tb._ntff_profile_via_ctypes("/opt/axon/libaxon_pjrt.so")
        mod = types.ModuleType("antenv.axon_hooks")
        mod._hook = hook
        mod.get_axon_ntff_profile_hook = lambda: mod._hook
        mod.set_axon_ntff_profile_hook = lambda h: None
        sys.modules["antenv.axon_hooks"] = mod
        antenv.axon_hooks = mod
    except Exception:
        pass


def _run_device(X, trace=False, **run_kwargs):
    from concourse.bass_utils import run_bass_kernel_spmd

    _install_ntff_hook()
    nc = _build()
    in_maps = []
    for c in range(N_CORES):
        shard = np.zeros((PAD_ROWS, P), dtype=np.float32)
        shard[:ROWS_PER_CORE] = X[c * ROWS_PER_CORE:(c + 1) * ROWS_PER_CORE]
        in_maps.append({"x": shard.reshape(CHUNKS, PART, FREE)})
    res = run_bass_kernel_spmd(
        nc, in_maps, list(range(N_CORES)), trace=trace, **run_kwargs
    )
    partials = np.stack([np.asarray(r["out"]).reshape(-1) for r in res.results])
    return partials, res


def _finish(partials, mu, W):
    s = partials.sum(axis=0, dtype=np.float64)   # (1152,)
    S1, S2 = s[:P], s[P:]
    n = float(N_ROWS)
    m = S1 / n
    mom2 = S2 / n - m * m
    cum = np.stack([m, np.zeros_like(m), mom2], axis=1).reshape(-1)  # (1728,)
    proj = (cum - mu.astype(np.float64)) @ W.astype(np.float64)
    return proj.astype(np.float32).reshape(1, PROJ_DIMS)


def kernel(X, mu, W):
    X = np.asarray(X, dtype=np.float32)
    mu = np.asarray(mu, dtype=np.float32)
    W = np.asarray(W, dtype=np.float32)
    partials, _ = _run_device(X)
    return _finish(partials, mu, W)


# revision 12
# speedup vs baseline: 1.1486x; 1.1486x over previous
"""Trainium2 Bass kernel for CumulantSOAP_CV.

reference math:
    m    = mean(X, axis=0)                       # (576,)
    mom1 = mean(X - m, axis=0)  (~0)             # (576,)
    mom2 = mean((X - m)^2, axis=0)               # (576,)
    cum  = interleave(m, mom1, mom2)             # (1, 1728)
    out  = (cum - mu) @ W                        # (1, 4)

Device kernel (SPMD over 8 cores, X sharded row-wise): each core streams its
(25000, 576) shard from HBM once and produces the per-shard column sums
S1 = sum(x) and S2 = sum(x^2) as a (1, 1152) output.  The tiny moment/
projection math on the 8 partial vectors happens on host in float64.

Per-core layout: the shard is zero-padded to 25088 rows and viewed as 28
chunks of (128 partitions x 4032 free), i.e. each partition holds 7
consecutive rows, so every chunk is a single contiguous 2.06 MB HBM read.
128 partitions matters: the HWDGE splits a DMA across SDMA engines by
divisibility of the partition count (125 partitions -> only 5 engines at
~23 GB/s each = 117 GB/s; 128 -> all 16 engines = full ~360 GB/s).
Column sums reduce over the partition axis with a ones(128,1).T @ tile bf16
matmul (1 cycle/row), accumulated in PSUM (fp32) across all row-groups.
X^2 comes from one ScalarE Square activation per chunk (bf16 out); the bf16
copy of X comes from one VectorE tensor_copy per chunk.  bf16 rounding of
the summands perturbs the final (1,4) projection by ~1e-6 relative -- far
inside tolerance.  Padded zero rows contribute nothing to either sum.
"""

import sys
import types

import numpy as np

N_CORES = 8
N_ROWS = 200000
P = 576
PROJ_DIMS = 4
ROWS_PER_CORE = N_ROWS // N_CORES   # 25000
PART = 128                          # SBUF partitions used per tile
RPP = 7                             # rows per partition per chunk
CHUNK_ROWS = PART * RPP             # 896
CHUNKS = 28                         # per core (28*896 = 25088, padded)
PAD_ROWS = CHUNKS * CHUNK_ROWS      # 25088
FREE = RPP * P                      # 4032
SPLIT = 288                         # 576 = 2x288; each half fits one PSUM bank


def _build(chunks=CHUNKS, part=PART, rpp=RPP):
    import concourse.bacc as bacc
    import concourse.mybir as mybir
    import concourse.tile as tile

    free = rpp * P
    nc = bacc.Bacc(None, target_bir_lowering=False)
    f32 = mybir.dt.float32
    bf16 = mybir.dt.bfloat16
    x = nc.dram_tensor("x", [chunks, part, free], f32, kind="ExternalInput")
    out = nc.dram_tensor("out", [1, 2 * P], f32, kind="ExternalOutput")

    with tile.TileContext(nc) as tc:
        with (
            tc.tile_pool(name="xp", bufs=6) as xp,
            tc.tile_pool(name="xmp", bufs=4) as xmp,
            tc.tile_pool(name="sqp", bufs=4) as sqp,
            tc.tile_pool(name="cst", bufs=1) as cst,
            tc.tile_pool(name="op", bufs=1) as op,
            tc.tile_pool(name="ps", bufs=1, space="PSUM") as ps,
        ):
            ones = cst.tile([part, 1], bf16)
            nc.vector.memset(ones[:], 1.0)
            # acc[0:2] = S1 halves, acc[2:4] = S2 halves
            acc = [
                ps.tile([1, SPLIT], f32, name=f"acc{i}", tag=f"acc{i}")
                for i in range(4)
            ]

            # Split the final chunk column-wise so the post-last-DMA compute
            # tail (cast + square + matmuls on a full 4032-wide tile) shrinks.
            pieces = [(c, 0, rpp) for c in range(chunks - 1)]
            pieces += [(chunks - 1, 0, rpp - 2), (chunks - 1, rpp - 2, rpp)]
            for c, g0, g1 in pieces:
                ng = g1 - g0
                w = ng * P
                xt = xp.tile([part, w], f32, name="xt", tag="xt")
                nc.sync.dma_start(out=xt[:], in_=x[c][:, g0 * P:g1 * P])
                # S1 path: bf16 cast on DVE, then all S1 matmuls.  Kept
                # separate from (and ahead of) the S2 path so the PE never
                # stalls on the slower ScalarE square: interleaving S1/S2 per
                # group made every chunk's matmul burst wait ~3.7us for the
                # full-tile ACTIVATE, HAM-throttling the PE to cold rate.
                xm = xmp.tile([part, w], bf16, name="xm", tag="xm")
                nc.vector.tensor_copy(xm[:], xt[:])
                for g in range(ng):
                    first = c == 0 and g0 == 0 and g == 0
                    last = c == chunks - 1 and g0 + g == rpp - 1
                    for h in range(2):
                        s = g * P + h * SPLIT
                        nc.tensor.matmul(
                            acc[h][:],
                            ones[:],
                            xm[:, s:s + SPLIT],
                            start=first,
                            stop=last,
                        )
                # S2 path: square in two half-tiles so the first half's
                # matmuls start ~2us before the whole square is done.
                mid = (ng + 1) // 2
                halves = [(0, mid, "sqa"), (mid, ng, "sqb")]
                for h0, h1, tag in halves:
                    if h0 == h1:
                        continue
                    sq = sqp.tile([part, (h1 - h0) * P], bf16, name=tag, tag=tag)
                    nc.scalar.square(sq[:], xt[:, h0 * P:h1 * P])
                    for g in range(h0, h1):
                        first = c == 0 and g0 == 0 and g == 0
                        last = c == chunks - 1 and g0 + g == rpp - 1
                        for h in range(2):
                            s = (g - h0) * P + h * SPLIT
                            nc.tensor.matmul(
                                acc[2 + h][:],
                                ones[:],
                                sq[:, s:s + SPLIT],
                                start=first,
                                stop=last,
                            )

            ot = op.tile([1, 2 * P], f32)
            for h in range(2):
                # S1 halves on DVE, S2 halves on ACT -- the four PSUM reads
                # run two-abreast instead of serially on one engine.
                nc.vector.tensor_copy(ot[:, h * SPLIT:(h + 1) * SPLIT], acc[h][:])
                nc.scalar.copy(
                    ot[:, P + h * SPLIT:P + (h + 1) * SPLIT], acc[2 + h][:]
                )
            nc.sync.dma_start(out=out[:], in_=ot[:])
    nc.compile()
    return nc


def _install_ntff_hook():
    """This image's antenv lacks axon_hooks, which bass_utils imports when
    tracing is requested (trace=True or BASS_TRACE=1).  Recreate the module
    from the injected libaxon_pjrt.so so tracing works instead of crashing.
    Harmless when tracing is off."""
    try:
        import antenv.axon_hooks  # noqa: F401
        return
    except ImportError:
        pass
    try:
        import antenv
        import trn_agent_boot.trn_boot as tb

        hook = tb._ntff_profile_via_ctypes("/opt/axon/libaxon_pjrt.so")
        mod = types.ModuleType("antenv.axon_hooks")
        mod._hook = hook
        mod.get_axon_ntff_profile_hook = lambda: mod._hook
        mod.set_axon_ntff_profile_hook = lambda h: None
        sys.modules["antenv.axon_hooks"] = mod
        antenv.axon_hooks = mod
    except Exception:
        pass


def _run_device(X, trace=False, **run_kwargs):
    from concourse.bass_utils import run_bass_kernel_spmd

    _install_ntff_hook()
    nc = _build()
    in_maps = []
    for c in range(N_CORES):
        shard = np.zeros((PAD_ROWS, P), dtype=np.float32)
        shard[:ROWS_PER_CORE] = X[c * ROWS_PER_CORE:(c + 1) * ROWS_PER_CORE]
        in_maps.append({"x": shard.reshape(CHUNKS, PART, FREE)})
    res = run_bass_kernel_spmd(
        nc, in_maps, list(range(N_CORES)), trace=trace, **run_kwargs
    )
    partials = np.stack([np.asarray(r["out"]).reshape(-1) for r in res.results])
    return partials, res


def _finish(partials, mu, W):
    s = partials.sum(axis=0, dtype=np.float64)   # (1152,)
    S1, S2 = s[:P], s[P:]
    n = float(N_ROWS)
    m = S1 / n
    mom2 = S2 / n - m * m
    cum = np.stack([m, np.zeros_like(m), mom2], axis=1).reshape(-1)  # (1728,)
    proj = (cum - mu.astype(np.float64)) @ W.astype(np.float64)
    return proj.astype(np.float32).reshape(1, PROJ_DIMS)


def kernel(X, mu, W):
    X = np.asarray(X, dtype=np.float32)
    mu = np.asarray(mu, dtype=np.float32)
    W = np.asarray(W, dtype=np.float32)
    partials, _ = _run_device(X)
    return _finish(partials, mu, W)
